# revision 15
# baseline (speedup 1.0000x reference)
"""Trainium2 Bass kernel: DiscreteEmbedding (rect-window embedding lookup).

Math (matches the jax reference bitwise):
    xs  = x * 2048;  y = xs + 0.5
    i_lo = ceil(y)-1, i_hi = floor(y)
    out[t] = 0.5*T[i_lo] + 0.5*T[i_hi]      (T extended with zero row 2048)
Non-boundary tokens (y non-integer): i_lo == i_hi -> out = T[i_lo].
Boundary tokens (y integer, ~1/4096 of tokens): out = avg of two rows.

Device strategy (8 cores, data-parallel over tokens):
  - Build a combined table TC in DRAM:
      TC[0:2048]    = T            (plain rows)
      TC[2048]      = 0            (i_lo == 2048 tail -> zero output)
      TC[2049+k]    = (T[k]+T[k+1])/2  for k<2048, with T[2048]=0
    and gather ONCE per token at idx2 = i_lo + 2049*b, b = (y integer).
    This halves GPSIMD descriptor-generation work vs a dual gather - the
    measured bottleneck (~10 ns/idx on one SWDGE queue).
  - 4 SWDGE queues, gather chunks round-robin -> parallel Q7 desc-gen.
  - x is passed wrapped [16,512] replicated to [128,512]: full-width DVE
    index math, and partitions 16..127 double as the per-Q7-core replicas
    of the int16 index buffer that dma_gather expects.
  - Gather output is position-ordered; stores are contiguous 1MB DMAs;
    host un-permutes rows (free) while un-sharding.
"""

import numpy as np

import concourse.bass as bass
import concourse.mybir as mybir
import concourse.tile as tile
from concourse import bacc, bass_utils

N_CORES = 8
B, S = 32, 2048
V, D = 2048, 128
TOK = B * S                 # 65536 tokens total
TPC = TOK // N_CORES        # 8192 tokens per core
SPC = TPC // 16             # 512: free dim of the wrapped [16, 512] x layout
# gather chunks: (j_blocks, swdge_queue). 64 j-blocks of 128 tokens total.
CHUNKS = [(2, 0), (21, 1), (21, 2), (20, 3)]
ABASE = V + 1               # 2049: base row of the averaged-pair table
VEXT = 4224                 # TC rows (>= 2*V+1, multiple of 128)
NQ = 4                      # SWDGE queues

F32 = mybir.dt.float32
I32 = mybir.dt.int32
I16 = mybir.dt.int16
OP = mybir.AluOpType


def build():
    nc = bacc.Bacc(
        "TRN2",
        target_bir_lowering=False,
        debug=False,
        num_devices=N_CORES,
        num_swdge_queues=NQ,
    )
    xr = nc.dram_tensor("xr", [128, SPC], F32, kind="ExternalInput")
    emb = nc.dram_tensor("emb", [V, D], F32, kind="ExternalInput")
    out = nc.dram_tensor("out", [TPC, D], F32, kind="ExternalOutput")
    tc_tbl = nc.dram_tensor("tc_tbl", [VEXT, D], F32, kind="Internal")

    with tile.TileContext(nc) as tc:
        with tc.tile_pool(name="sb", bufs=1) as sb, tc.tile_pool(name="g", bufs=1) as gp:
            # ---- x load first so index math runs during table prep ----
            xt = sb.tile([128, SPC], F32)
            nc.sync.dma_start(out=xt[:], in_=xr[:])

            # ---- combined table TC = [T; 0; avg-pairs] ----
            tbl = sb.tile([128, (V // 128) * D], F32)   # T rows, 16 rows/partition
            nc.sync.dma_start(
                out=tbl[:], in_=emb[:].rearrange("(p n) d -> p (n d)", p=128)
            )
            # avg[k] = (T[k]+T[k+1])/2.  Within a partition (rows 16p..16p+15)
            # the +1 shift is a free-dim offset; the n=15 element needs the
            # next partition's first row (tnext), fetched by a tiny
            # SBUF->SBUF partition-shifted DMA; last partition pads zero.
            tnext = sb.tile([128, D], F32)
            nc.vector.memset(tnext[:], 0.0)
            # tnext[p] = T[16(p+1)] straight from DRAM (strided rows 16,32,..2032)
            nc.sync.dma_start(
                out=tnext[0:127, :],
                in_=emb[16:V].rearrange("(p n) d -> p (n d)", p=127)[:, 0:D],
            )
            avg = sb.tile([128, (V // 128) * D], F32)
            nc.vector.tensor_add(
                avg[:, 0 : 15 * D], tbl[:, 0 : 15 * D], tbl[:, D : 16 * D]
            )
            nc.vector.tensor_add(avg[:, 15 * D : 16 * D], tbl[:, 15 * D : 16 * D], tnext[:])
            nc.vector.tensor_scalar_mul(avg[:], avg[:], 0.5)
            zrow = sb.tile([1, D], F32)
            nc.vector.memset(zrow[:], 0.0)
            nc.scalar.dma_start(
                out=tc_tbl[0:V].rearrange("(p n) d -> p (n d)", p=128), in_=tbl[:]
            )
            nc.scalar.dma_start(out=tc_tbl[V : V + 1, :], in_=zrow[:])
            nc.scalar.dma_start(
                out=tc_tbl[ABASE : ABASE + V].rearrange("(p n) d -> p (n d)", p=128),
                in_=avg[:],
            )

            # ---- index math (fp32, exact): y = x*2048 + 0.5 ----
            y = sb.tile([128, SPC], F32)
            nc.vector.tensor_scalar(y[:], xt[:], 2048.0, 0.5, op0=OP.mult, op1=OP.add)
            # i0 = int(y) rounded to SOME neighboring integer; fix up with
            # exact fp32 compares (robust to the HW float->int round mode).
            i0 = sb.tile([128, SPC], I32)
            nc.vector.tensor_copy(i0[:], y[:])
            f0 = sb.tile([128, SPC], F32)
            nc.vector.tensor_copy(f0[:], i0[:])
            lt = sb.tile([128, SPC], F32)    # f0 < y
            nc.vector.tensor_tensor(lt[:], f0[:], y[:], op=OP.is_lt)
            bnd = sb.tile([128, SPC], F32)   # y integer -> blend row
            nc.vector.tensor_tensor(bnd[:], f0[:], y[:], op=OP.is_equal)
            lf = sb.tile([128, SPC], F32)    # i_lo = ceil(y) - 1
            nc.vector.tensor_add(lf[:], f0[:], lt[:])
            nc.vector.tensor_scalar_add(lf[:], lf[:], -1.0)
            # idx2 = i_lo + 2049*b
            idxf = sb.tile([128, SPC], F32)
            nc.vector.scalar_tensor_tensor(
                out=idxf[:],
                in0=bnd[:],
                scalar=float(ABASE),
                in1=lf[:],
                op0=OP.mult,
                op1=OP.add,
            )
            idx16 = sb.tile([128, SPC], I16)
            nc.vector.tensor_copy(idx16[:], idxf[:])

            # ---- chunked gather + store ----
            # First chunk tiny: the first SWDGE op blocks the Pool engine for
            # its whole desc-gen; later ones dispatch async and the queues
            # generate concurrently.
            out_v = out[:].rearrange("(p j) d -> p (j d)", p=128)
            j0 = 0
            for ci, (jbc, q) in enumerate(CHUNKS):
                g = gp.tile([128, jbc * D], F32, tag=f"g{ci}")
                nc.gpsimd.dma_gather(
                    g[:].rearrange("p (j d) -> p j d", d=D),
                    tc_tbl[0 : ABASE + V],
                    idx16[:, j0 * 8 : (j0 + jbc) * 8],
                    num_idxs=128 * jbc,
                    num_idxs_reg=128 * jbc,
                    elem_size=D,
                    single_packet=False,
                    queue_num=q,
                )
                nc.sync.dma_start(
                    out=out_v[:, j0 * D : (j0 + jbc) * D], in_=g[:]
                )
                j0 += jbc
            assert j0 == TPC // 128
    nc.compile()
    return nc


_NC = None


def _row_perm():
    """out row r holds gather position i(r); position i handles token
    t(i) = (i%16)*512 + i//16 (x wrapped [16,512] across partitions)."""
    r = np.arange(TPC)
    p, j = r // 64, r % 64
    i = j * 128 + p
    return (i % 16) * SPC + i // 16  # token index held at row r


def kernel(x, time_embedding):
    global _NC
    x = np.ascontiguousarray(np.asarray(x, dtype=np.float32))
    t = np.ascontiguousarray(np.asarray(time_embedding, dtype=np.float32))
    xf = x.reshape(-1)
    in_maps = []
    for c in range(N_CORES):
        xc = xf[c * TPC : (c + 1) * TPC].reshape(16, SPC)
        in_maps.append({"xr": np.ascontiguousarray(np.tile(xc, (8, 1))), "emb": t})

    if _NC is None:
        _NC = build()
    res = bass_utils.run_bass_kernel_spmd(_NC, in_maps, core_ids=list(range(N_CORES)))
    global _LAST_RES
    _LAST_RES = res

    tkn = _row_perm()
    outs = []
    for c in range(N_CORES):
        oc = np.asarray(res.results[c]["out"])
        full = np.empty_like(oc)
        full[tkn] = oc
        outs.append(full)
    return np.concatenate(outs, axis=0).reshape(B, S, D)


# revision 16
# speedup vs baseline: 1.0320x; 1.0320x over previous
"""Trainium2 Bass kernel: DiscreteEmbedding (rect-window embedding lookup).

Math (matches the jax reference bitwise):
    xs  = x * 2048;  y = xs + 0.5
    i_lo = ceil(y)-1, i_hi = floor(y)
    out[t] = 0.5*T[i_lo] + 0.5*T[i_hi]      (T extended with zero row 2048)
Non-boundary tokens (y non-integer): i_lo == i_hi -> out = T[i_lo].
Boundary tokens (y integer, ~1/4096 of tokens): out = avg of two rows.

Device strategy (8 cores, data-parallel over tokens):
  - Build a combined table TC in DRAM:
      TC[0:2048]    = T            (plain rows)
      TC[2048]      = 0            (i_lo == 2048 tail -> zero output)
      TC[2049+k]    = (T[k]+T[k+1])/2  for k<2048, with T[2048]=0
    and gather ONCE per token at idx2 = i_lo + 2049*b, b = (y integer).
    This halves GPSIMD descriptor-generation work vs a dual gather - the
    measured bottleneck (~10 ns/idx on one SWDGE queue).
  - 4 SWDGE queues, gather chunks round-robin -> parallel Q7 desc-gen.
  - x is passed wrapped [16,512] replicated to [128,512]: full-width DVE
    index math, and partitions 16..127 double as the per-Q7-core replicas
    of the int16 index buffer that dma_gather expects.
  - Gather output is position-ordered; stores are contiguous 1MB DMAs;
    host un-permutes rows (free) while un-sharding.
"""

import numpy as np

import concourse.bass as bass
import concourse.mybir as mybir
import concourse.tile as tile
from concourse import bacc, bass_utils

N_CORES = 8
B, S = 32, 2048
V, D = 2048, 128
TOK = B * S                 # 65536 tokens total
TPC = TOK // N_CORES        # 8192 tokens per core
SPC = TPC // 16             # 512: free dim of the wrapped [16, 512] x layout
# gather chunks: (j_blocks, swdge_queue). 64 j-blocks of 128 tokens total.
CHUNKS = [(2, 0), (15, 1), (16, 2), (16, 3), (15, 0)]
ABASE = V + 1               # 2049: base row of the averaged-pair table
VEXT = 4224                 # TC rows (>= 2*V+1, multiple of 128)
NQ = 4                      # SWDGE queues

F32 = mybir.dt.float32
I32 = mybir.dt.int32
I16 = mybir.dt.int16
OP = mybir.AluOpType


def build():
    nc = bacc.Bacc(
        "TRN2",
        target_bir_lowering=False,
        debug=False,
        num_devices=N_CORES,
        num_swdge_queues=NQ,
    )
    xr = nc.dram_tensor("xr", [128, SPC], F32, kind="ExternalInput")
    emb = nc.dram_tensor("emb", [V, D], F32, kind="ExternalInput")
    out = nc.dram_tensor("out", [TPC, D], F32, kind="ExternalOutput")
    tc_tbl = nc.dram_tensor("tc_tbl", [VEXT, D], F32, kind="Internal")

    with tile.TileContext(nc) as tc:
        with tc.tile_pool(name="sb", bufs=1) as sb, tc.tile_pool(name="g", bufs=1) as gp:
            # ---- x load first so index math runs during table prep ----
            xt = sb.tile([128, SPC], F32)
            nc.sync.dma_start(out=xt[:], in_=xr[:])

            # ---- combined table TC = [T; 0; avg-pairs] ----
            tbl = sb.tile([128, (V // 128) * D], F32)   # T rows, 16 rows/partition
            nc.sync.dma_start(
                out=tbl[:], in_=emb[:].rearrange("(p n) d -> p (n d)", p=128)
            )
            # avg[k] = (T[k]+T[k+1])/2.  Within a partition (rows 16p..16p+15)
            # the +1 shift is a free-dim offset; the n=15 element needs the
            # next partition's first row (tnext), fetched by a tiny
            # SBUF->SBUF partition-shifted DMA; last partition pads zero.
            tnext = sb.tile([128, D], F32)
            nc.vector.memset(tnext[:], 0.0)
            # tnext[p] = T[16(p+1)] straight from DRAM (strided rows 16,32,..2032)
            nc.sync.dma_start(
                out=tnext[0:127, :],
                in_=emb[16:V].rearrange("(p n) d -> p (n d)", p=127)[:, 0:D],
            )
            avg = sb.tile([128, (V // 128) * D], F32)
            nc.vector.tensor_add(
                avg[:, 0 : 15 * D], tbl[:, 0 : 15 * D], tbl[:, D : 16 * D]
            )
            nc.vector.tensor_add(avg[:, 15 * D : 16 * D], tbl[:, 15 * D : 16 * D], tnext[:])
            nc.vector.tensor_scalar_mul(avg[:], avg[:], 0.5)
            zrow = sb.tile([1, D], F32)
            nc.vector.memset(zrow[:], 0.0)
            nc.scalar.dma_start(
                out=tc_tbl[0:V].rearrange("(p n) d -> p (n d)", p=128), in_=tbl[:]
            )
            nc.scalar.dma_start(out=tc_tbl[V : V + 1, :], in_=zrow[:])
            nc.scalar.dma_start(
                out=tc_tbl[ABASE : ABASE + V].rearrange("(p n) d -> p (n d)", p=128),
                in_=avg[:],
            )

            # ---- index math (fp32, exact): y = x*2048 + 0.5 ----
            y = sb.tile([128, SPC], F32)
            nc.vector.tensor_scalar(y[:], xt[:], 2048.0, 0.5, op0=OP.mult, op1=OP.add)
            # i0 = int(y) rounded to SOME neighboring integer; fix up with
            # exact fp32 compares (robust to the HW float->int round mode).
            i0 = sb.tile([128, SPC], I32)
            nc.vector.tensor_copy(i0[:], y[:])
            f0 = sb.tile([128, SPC], F32)
            nc.vector.tensor_copy(f0[:], i0[:])
            lt = sb.tile([128, SPC], F32)    # f0 < y
            nc.vector.tensor_tensor(lt[:], f0[:], y[:], op=OP.is_lt)
            bnd = sb.tile([128, SPC], F32)   # y integer -> blend row
            nc.vector.tensor_tensor(bnd[:], f0[:], y[:], op=OP.is_equal)
            lf = sb.tile([128, SPC], F32)    # i_lo = ceil(y) - 1
            nc.vector.tensor_add(lf[:], f0[:], lt[:])
            nc.vector.tensor_scalar_add(lf[:], lf[:], -1.0)
            # idx2 = i_lo + 2049*b
            idxf = sb.tile([128, SPC], F32)
            nc.vector.scalar_tensor_tensor(
                out=idxf[:],
                in0=bnd[:],
                scalar=float(ABASE),
                in1=lf[:],
                op0=OP.mult,
                op1=OP.add,
            )
            idx16 = sb.tile([128, SPC], I16)
            nc.vector.tensor_copy(idx16[:], idxf[:])

            # ---- chunked gather + store ----
            # First chunk tiny: the first SWDGE op blocks the Pool engine for
            # its whole desc-gen; later ones dispatch async and the queues
            # generate concurrently.
            out_v = out[:].rearrange("(p j) d -> p (j d)", p=128)
            j0 = 0
            for ci, (jbc, q) in enumerate(CHUNKS):
                g = gp.tile([128, jbc * D], F32, tag=f"g{ci}")
                nc.gpsimd.dma_gather(
                    g[:].rearrange("p (j d) -> p j d", d=D),
                    tc_tbl[0 : ABASE + V],
                    idx16[:, j0 * 8 : (j0 + jbc) * 8],
                    num_idxs=128 * jbc,
                    num_idxs_reg=128 * jbc,
                    elem_size=D,
                    single_packet=False,
                    queue_num=q,
                )
                nc.sync.dma_start(
                    out=out_v[:, j0 * D : (j0 + jbc) * D], in_=g[:]
                )
                j0 += jbc
            assert j0 == TPC // 128
    nc.compile()
    return nc


_NC = None


def _row_perm():
    """out row r holds gather position i(r); position i handles token
    t(i) = (i%16)*512 + i//16 (x wrapped [16,512] across partitions)."""
    r = np.arange(TPC)
    p, j = r // 64, r % 64
    i = j * 128 + p
    return (i % 16) * SPC + i // 16  # token index held at row r


def kernel(x, time_embedding):
    global _NC
    x = np.ascontiguousarray(np.asarray(x, dtype=np.float32))
    t = np.ascontiguousarray(np.asarray(time_embedding, dtype=np.float32))
    xf = x.reshape(-1)
    in_maps = []
    for c in range(N_CORES):
        xc = xf[c * TPC : (c + 1) * TPC].reshape(16, SPC)
        in_maps.append({"xr": np.ascontiguousarray(np.tile(xc, (8, 1))), "emb": t})

    if _NC is None:
        _NC = build()
    res = bass_utils.run_bass_kernel_spmd(_NC, in_maps, core_ids=list(range(N_CORES)))
    global _LAST_RES
    _LAST_RES = res

    tkn = _row_perm()
    outs = []
    for c in range(N_CORES):
        oc = np.asarray(res.results[c]["out"])
        full = np.empty_like(oc)
        full[tkn] = oc
        outs.append(full)
    return np.concatenate(outs, axis=0).reshape(B, S, D)


# revision 17
# speedup vs baseline: 1.1760x; 1.1396x over previous
"""Trainium2 Bass kernel: DiscreteEmbedding (rect-window embedding lookup).

Math (matches the jax reference bitwise):
    xs  = x * 2048;  y = xs + 0.5
    i_lo = ceil(y)-1, i_hi = floor(y)
    out[t] = 0.5*T[i_lo] + 0.5*T[i_hi]      (T extended with zero row 2048)
Non-boundary tokens (y non-integer): i_lo == i_hi -> out = T[i_lo].
Boundary tokens (y integer, ~1/4096 of tokens): out = avg of two rows.

Device strategy (8 cores, data-parallel over tokens):
  - Build a combined table TC in DRAM:
      TC[0:2048]    = T            (plain rows)
      TC[2048]      = 0            (i_lo == 2048 tail -> zero output)
      TC[2049+k]    = (T[k]+T[k+1])/2  for k<2048, with T[2048]=0
    and gather ONCE per token at idx2 = i_lo + 2049*b, b = (y integer).
    This halves GPSIMD descriptor-generation work vs a dual gather - the
    measured bottleneck (~10 ns/idx on one SWDGE queue).
  - 4 SWDGE queues, gather chunks round-robin -> parallel Q7 desc-gen.
  - x is passed wrapped [16,512] replicated to [128,512]: full-width DVE
    index math, and partitions 16..127 double as the per-Q7-core replicas
    of the int16 index buffer that dma_gather expects.
  - Gather output is position-ordered; stores are contiguous 1MB DMAs;
    host un-permutes rows (free) while un-sharding.
"""

import numpy as np

import concourse.bass as bass
import concourse.mybir as mybir
import concourse.tile as tile
from concourse import bacc, bass_utils

N_CORES = 8
B, S = 32, 2048
V, D = 2048, 128
TOK = B * S                 # 65536 tokens total
TPC = TOK // N_CORES        # 8192 tokens per core
SPC = TPC // 16             # 512: free dim of the wrapped [16, 512] x layout
# gather chunks: (j_blocks, swdge_queue). 64 j-blocks of 128 tokens total.
CHUNKS = [(2, 0), (8, 1), (8, 2), (8, 3), (8, 0), (8, 1), (8, 2), (7, 3), (7, 0)]
ABASE = V + 1               # 2049: base row of the averaged-pair table
VEXT = 4224                 # TC rows (>= 2*V+1, multiple of 128)
NQ = 4                      # SWDGE queues

F32 = mybir.dt.float32
I32 = mybir.dt.int32
I16 = mybir.dt.int16
OP = mybir.AluOpType


def build():
    nc = bacc.Bacc(
        "TRN2",
        target_bir_lowering=False,
        debug=False,
        num_devices=N_CORES,
        num_swdge_queues=NQ,
    )
    xr = nc.dram_tensor("xr", [128, SPC], F32, kind="ExternalInput")
    emb = nc.dram_tensor("emb", [V, D], F32, kind="ExternalInput")
    out = nc.dram_tensor("out", [TPC, D], F32, kind="ExternalOutput")
    tc_tbl = nc.dram_tensor("tc_tbl", [VEXT, D], F32, kind="Internal")

    with tile.TileContext(nc) as tc:
        with tc.tile_pool(name="sb", bufs=1) as sb, tc.tile_pool(name="g", bufs=1) as gp:
            # ---- x load first so index math runs during table prep ----
            xt = sb.tile([128, SPC], F32)
            nc.sync.dma_start(out=xt[:], in_=xr[:])

            # ---- combined table TC = [T; 0; avg-pairs] ----
            tbl = sb.tile([128, (V // 128) * D], F32)   # T rows, 16 rows/partition
            nc.sync.dma_start(
                out=tbl[:], in_=emb[:].rearrange("(p n) d -> p (n d)", p=128)
            )
            # avg[k] = (T[k]+T[k+1])/2.  Within a partition (rows 16p..16p+15)
            # the +1 shift is a free-dim offset; the n=15 element needs the
            # next partition's first row (tnext), fetched by a tiny
            # SBUF->SBUF partition-shifted DMA; last partition pads zero.
            tnext = sb.tile([128, D], F32)
            nc.vector.memset(tnext[:], 0.0)
            # tnext[p] = T[16(p+1)] straight from DRAM (strided rows 16,32,..2032)
            nc.sync.dma_start(
                out=tnext[0:127, :],
                in_=emb[16:V].rearrange("(p n) d -> p (n d)", p=127)[:, 0:D],
            )
            avg = sb.tile([128, (V // 128) * D], F32)
            nc.vector.tensor_add(
                avg[:, 0 : 15 * D], tbl[:, 0 : 15 * D], tbl[:, D : 16 * D]
            )
            nc.vector.tensor_add(avg[:, 15 * D : 16 * D], tbl[:, 15 * D : 16 * D], tnext[:])
            nc.vector.tensor_scalar_mul(avg[:], avg[:], 0.5)
            zrow = sb.tile([1, D], F32)
            nc.vector.memset(zrow[:], 0.0)
            nc.scalar.dma_start(
                out=tc_tbl[0:V].rearrange("(p n) d -> p (n d)", p=128), in_=tbl[:]
            )
            nc.scalar.dma_start(out=tc_tbl[V : V + 1, :], in_=zrow[:])
            nc.scalar.dma_start(
                out=tc_tbl[ABASE : ABASE + V].rearrange("(p n) d -> p (n d)", p=128),
                in_=avg[:],
            )

            # ---- index math (fp32, exact): y = x*2048 + 0.5 ----
            y = sb.tile([128, SPC], F32)
            nc.vector.tensor_scalar(y[:], xt[:], 2048.0, 0.5, op0=OP.mult, op1=OP.add)
            # i0 = int(y) rounded to SOME neighboring integer; fix up with
            # exact fp32 compares (robust to the HW float->int round mode).
            i0 = sb.tile([128, SPC], I32)
            nc.vector.tensor_copy(i0[:], y[:])
            f0 = sb.tile([128, SPC], F32)
            nc.vector.tensor_copy(f0[:], i0[:])
            lt = sb.tile([128, SPC], F32)    # f0 < y
            nc.vector.tensor_tensor(lt[:], f0[:], y[:], op=OP.is_lt)
            bnd = sb.tile([128, SPC], F32)   # y integer -> blend row
            nc.vector.tensor_tensor(bnd[:], f0[:], y[:], op=OP.is_equal)
            lf = sb.tile([128, SPC], F32)    # i_lo = ceil(y) - 1
            nc.vector.tensor_add(lf[:], f0[:], lt[:])
            nc.vector.tensor_scalar_add(lf[:], lf[:], -1.0)
            # idx2 = i_lo + 2049*b
            idxf = sb.tile([128, SPC], F32)
            nc.vector.scalar_tensor_tensor(
                out=idxf[:],
                in0=bnd[:],
                scalar=float(ABASE),
                in1=lf[:],
                op0=OP.mult,
                op1=OP.add,
            )
            idx16 = sb.tile([128, SPC], I16)
            nc.vector.tensor_copy(idx16[:], idxf[:])

            # ---- chunked gather + store ----
            # First chunk tiny: the first SWDGE op blocks the Pool engine for
            # its whole desc-gen; later ones dispatch async and the queues
            # generate concurrently.
            out_v = out[:].rearrange("(p j) d -> p (j d)", p=128)
            j0 = 0
            for ci, (jbc, q) in enumerate(CHUNKS):
                g = gp.tile([128, jbc * D], F32, tag=f"g{ci}")
                nc.gpsimd.dma_gather(
                    g[:].rearrange("p (j d) -> p j d", d=D),
                    tc_tbl[0 : ABASE + V],
                    idx16[:, j0 * 8 : (j0 + jbc) * 8],
                    num_idxs=128 * jbc,
                    num_idxs_reg=128 * jbc,
                    elem_size=D,
                    single_packet=False,
                    queue_num=q,
                )
                nc.sync.dma_start(
                    out=out_v[:, j0 * D : (j0 + jbc) * D], in_=g[:]
                )
                j0 += jbc
            assert j0 == TPC // 128
    nc.compile()
    return nc


_NC = None


def _row_perm():
    """out row r holds gather position i(r); position i handles token
    t(i) = (i%16)*512 + i//16 (x wrapped [16,512] across partitions)."""
    r = np.arange(TPC)
    p, j = r // 64, r % 64
    i = j * 128 + p
    return (i % 16) * SPC + i // 16  # token index held at row r


def kernel(x, time_embedding):
    global _NC
    x = np.ascontiguousarray(np.asarray(x, dtype=np.float32))
    t = np.ascontiguousarray(np.asarray(time_embedding, dtype=np.float32))
    xf = x.reshape(-1)
    in_maps = []
    for c in range(N_CORES):
        xc = xf[c * TPC : (c + 1) * TPC].reshape(16, SPC)
        in_maps.append({"xr": np.ascontiguousarray(np.tile(xc, (8, 1))), "emb": t})

    if _NC is None:
        _NC = build()
    res = bass_utils.run_bass_kernel_spmd(_NC, in_maps, core_ids=list(range(N_CORES)))
    global _LAST_RES
    _LAST_RES = res

    tkn = _row_perm()
    outs = []
    for c in range(N_CORES):
        oc = np.asarray(res.results[c]["out"])
        full = np.empty_like(oc)
        full[tkn] = oc
        outs.append(full)
    return np.concatenate(outs, axis=0).reshape(B, S, D)
